# revision 30
# baseline (speedup 1.0000x reference)
"""AtomEmbedding kernel for 8 TRN2 NeuronCores.

Reference semantics: rank-remap of atom types through the sorted unique
values present in the batch, then embedding lookup:
    uniq = unique(atom_types)  (padded sorted)
    out[b, a] = embedding[searchsorted(uniq, atom_types[b, a])]

Device strategy (data-parallel over n_batch): the lookup is computed as a
one-hot matmul on the TensorEngine. v9 pipeline per core:

  per pair p of super-batches (8192 atoms = 16 chunks of 512):
    SYNC  : one broadcast DMA [100, 8192] HBM->SBUF replicating the idx
            bytes (10 DRAM copies x 10-way fanout, 8 KB descriptors,
            4-deep ring so the next transfers hide under compute)
  per 4-chunk unit (2048 atoms; wide ops amortize the ~210 ns DVE
  instruction overhead):
    DVE   : oh[t, q] = (rep == iota_t)   bf16 one-hot, rows 0-99.
            Rows 100-127 are zeroed once at startup and never rewritten,
            so the 128-row weight slices are exact; the 128-row
            contraction enables the PE's Fast Weight Load
            (EnableFWL requires NumWeights==128).
  per chunk k (512 atoms):
    PE    : out[q, d] = oh[:, s*128:].T @ tbl_bf16[128, 64] -> PSUM
  per super-batch sb (8 chunks):
    ACT   : 2 copies [128, 1024] PSUM(f32)->SBUF(bf16)
    SYNC  : one 0.5 MB DMA SBUF->DRAM (bf16; last one split for tail
            overlap).

The one-hot matmul selects exactly one bf16 table row per atom (no
accumulation), so the f32 PSUM values are bf16-exact and the bf16
writeback is lossless; the host upcast restores identical f32 values.

DMA-completion semaphores are PER RING SLOT: increments from distinct
in-flight DMAs interleave across the 16 SDMA engines, so a single
cumulative counter can reach its threshold before any one DMA finished.
Engine-produced increments (oh_d, wbA, pout_rdy) complete in program
order, so cumulative counters are safe there.

Raw-bass engine blocks with standalone wait_ge (the neuronxcc walrus in
this toolchain cannot encode multi-wait sync on one instruction, nor
InstPartitionBroadcast).

Self-contained: shapes hardcoded, no sibling imports.
"""

import sys

if "/opt/trn_rl_repo" not in sys.path:
    sys.path.insert(0, "/opt/trn_rl_repo")

import numpy as np

N_BATCH = 9000
ATOMS_PER_MOL = 64
EMBED_DIM = 64
NUM_TYPES = 100
N_CORES = 8

ROWS_PER_CORE = N_BATCH * ATOMS_PER_MOL // N_CORES  # 72000
T_SLOTS = 576  # padded slots per partition
PAD_ROWS = 128 * T_SLOTS  # 73728
N_CHUNKS = T_SLOTS // 4  # 144 chunks of 512 atoms
N_SB = N_CHUNKS // 8  # 18 super-batches of 4096 atoms
N_PAIR = N_SB // 2  # 9 replicate pairs of 8192 atoms
N_COPIES = 10  # idx feed copies in DRAM (fanout 10 per copy)
PR_W = 8192  # pair atoms
RING = 6  # replicate ring depth (pairs)

_CACHE = {}


def _build_graph():
    import concourse.bass as bass
    import concourse.mybir as mybir

    f32 = mybir.dt.float32
    bf16 = mybir.dt.bfloat16
    AF = mybir.ActivationFunctionType
    OP = mybir.AluOpType

    nc = bass.Bass()

    i8 = mybir.dt.int8
    idx_d = nc.declare_dram_parameter(
        "idx", [N_COPIES, N_CHUNKS * 512], i8, isOutput=False
    )
    iota_d = nc.declare_dram_parameter("iota", [128, 1], f32, isOutput=False)
    tbl_d = nc.declare_dram_parameter("tbl", [128, 64], bf16, isOutput=False)
    out_d = nc.declare_dram_parameter(
        "out", [PAD_ROWS, EMBED_DIM], bf16, isOutput=True
    )

    from contextlib import ExitStack

    with ExitStack() as stack:
        iota_sb = stack.enter_context(nc.sbuf_tensor("iota_sb", [128, 1], f32))
        tbl_sb = stack.enter_context(nc.sbuf_tensor("tbl_sb", [128, 64], bf16))
        rep_sb = stack.enter_context(
            nc.sbuf_tensor("rep_sb", [100, RING * PR_W], i8)
        )
        oh_sb = stack.enter_context(nc.sbuf_tensor("oh_sb", [128, 16 * 512], bf16))
        outb_sb = stack.enter_context(
            nc.sbuf_tensor("outb_sb", [128, 6 * 2048], bf16)
        )
        pout = [
            stack.enter_context(nc.psum_tensor(f"pout{_i}", [128, 2048], f32))
            for _i in range(2)
        ]
        in1 = stack.enter_context(nc.semaphore("in1"))
        in2 = stack.enter_context(nc.semaphore("in2"))
        rep_h = stack.enter_context(nc.semaphore("rep_h"))
        rep_p = [stack.enter_context(nc.semaphore(f"rep_p{i}")) for i in range(RING)]
        oh_d = stack.enter_context(nc.semaphore("oh_d"))
        pout_rdy = stack.enter_context(nc.semaphore("pout_rdy"))
        wbA = stack.enter_context(nc.semaphore("wbA"))
        wbd = [stack.enter_context(nc.semaphore(f"wbd{i}")) for i in range(6)]
        block = stack.enter_context(nc.Block())
        iota_ap = iota_sb[:100, 0:1]
        out_flat = out_d[:].rearrange("(p t) d -> p (t d)", p=128)

        def ohv(k):
            return oh_sb[:, (k % 16) * 512 : (k % 16) * 512 + 512]

        def rep_src_ap(off, w):
            # 10 DRAM copies x 10-way 0-stride fanout onto 100 partitions
            return bass.AP(
                idx_d[:].tensor,
                off,
                [[N_CHUNKS * 512, N_COPIES], [0, 100 // N_COPIES], [1, w]],
            )

        def region_waits(eng, k, seen):
            # gate on the rep DMA covering chunk k's column region
            p, h = k // 16, (k % 16) // 8
            if (p, h) in seen:
                return
            seen.add((p, h))
            if p == 0 and h == 0:
                eng.wait_ge(rep_h, 16)
            elif p == 0:
                eng.wait_ge(rep_p[0], 16)
            else:
                eng.wait_ge(rep_p[p % RING], 16 * (p // RING + 1))

        @block.vector
        def _(dve):
            seen = set()
            # one-hot rows 100-127 are never written by the compares; zero
            # them once so the 128-row (FWL-enabling) weight reads are exact
            dve.memset(oh_sb[96:128, :], 0.0)
            dve.wait_ge(in1, 16)
            N_UNIT = N_CHUNKS // 8  # 8-chunk compare units (one half-pair)
            for u in range(N_UNIT):
                region_waits(dve, 8 * u, seen)
                if u >= 2:
                    dve.wait_ge(pout_rdy, 8 * (u - 1))  # PE freed oh unit u-2
                p = u // 2
                base = (p % RING) * PR_W + (u % 2) * 4096
                dve.tensor_scalar(
                    out=oh_sb[:100, (u % 2) * 4096 : (u % 2) * 4096 + 4096],
                    in0=rep_sb[:, base : base + 4096],
                    scalar1=iota_ap,
                    scalar2=None,
                    op0=OP.is_equal,
                ).then_inc(oh_d, 1)

        @block.tensor
        def _(te):
            te.wait_ge(in2, 16)
            for k in range(N_CHUNKS):
                sb = k // 8
                if k % 8 == 0 and sb >= 2:
                    te.wait_ge(wbA, 2 * (sb - 1))  # ACT copied both halves of sb-2
                te.wait_ge(oh_d, k // 8 + 1)
                for s in range(4):
                    mm = te.matmul(
                        out=pout[sb % 2][
                            :, (k % 8) * 256 + s * 64 : (k % 8) * 256 + (s + 1) * 64
                        ],
                        lhsT=ohv(k)[:, s * 128 : (s + 1) * 128],
                        rhs=tbl_sb[:128, :],
                        start=True,
                        stop=True,
                    )
                    if s == 3:
                        mm.then_inc(pout_rdy, 1)

        @block.scalar
        def _(act):
            act.dma_start(out=iota_sb[:], in_=iota_d[:]).then_inc(in1, 16)
            act.dma_start(out=tbl_sb[:], in_=tbl_d[:]).then_inc(in2, 16)
            for sb in range(N_SB):
                # outb slot free (out DMA of sb-6 done)
                if sb >= 6:
                    act.wait_ge(wbd[sb % 6], 16 * (sb // 6))
                act.wait_ge(pout_rdy, 8 * sb + 4)
                act.activation(
                    out=outb_sb[:, (sb % 6) * 2048 : (sb % 6) * 2048 + 1024],
                    in_=pout[sb % 2][:, :1024],
                    func=AF.Copy,
                ).then_inc(wbA, 1)
                act.wait_ge(pout_rdy, 8 * (sb + 1))
                act.activation(
                    out=outb_sb[:, (sb % 6) * 2048 + 1024 : (sb % 6) * 2048 + 2048],
                    in_=pout[sb % 2][:, 1024:],
                    func=AF.Copy,
                ).then_inc(wbA, 1)

        @block.sync
        def _(sync):
            def issue_rep(p):
                if p >= RING:
                    sync.wait_ge(oh_d, 2 * (p - RING + 1))  # pair p-RING consumed
                sync.dma_start(
                    out=rep_sb[:, (p % RING) * PR_W : (p % RING) * PR_W + PR_W],
                    in_=rep_src_ap(p * PR_W, PR_W),
                ).then_inc(rep_p[p % RING], 16)

            # pair 0 split in half so compares can start sooner; even
            # prologue pairs follow on this ring
            sync.dma_start(
                out=rep_sb[:, 0 : PR_W // 2],
                in_=rep_src_ap(0, PR_W // 2),
            ).then_inc(rep_h, 16)
            sync.dma_start(
                out=rep_sb[:, PR_W // 2 : PR_W],
                in_=rep_src_ap(PR_W // 2, PR_W // 2),
            ).then_inc(rep_p[0], 16)
            for p in range(1, RING):
                issue_rep(p)
            for sb in range(N_SB):
                if sb % 2 == 0 and sb // 2 + RING < N_PAIR:
                    issue_rep(sb // 2 + RING)
                if sb < N_SB - 1:
                    sync.wait_ge(wbA, 2 * (sb + 1))
                    sync.dma_start(
                        out=out_flat[:, sb * 2048 : (sb + 1) * 2048],
                        in_=outb_sb[:, (sb % 6) * 2048 : (sb % 6) * 2048 + 2048],
                    ).then_inc(wbd[sb % 6], 16)
                else:
                    # split the last writeback so its two halves overlap the
                    # final ACT copies
                    sync.wait_ge(wbA, 2 * sb + 1)
                    sync.dma_start(
                        out=out_flat[:, sb * 2048 : sb * 2048 + 1024],
                        in_=outb_sb[:, (sb % 6) * 2048 : (sb % 6) * 2048 + 1024],
                    ).then_inc(wbd[sb % 6], 16)
                    sync.wait_ge(wbA, 2 * (sb + 1))
                    sync.dma_start(
                        out=out_flat[:, sb * 2048 + 1024 : (sb + 1) * 2048],
                        in_=outb_sb[
                            :, (sb % 6) * 2048 + 1024 : (sb % 6) * 2048 + 2048
                        ],
                    ).then_inc(wbd[sb % 6], 16)
            for i in range(6):
                sync.wait_ge(wbd[i], 64 if i == 5 else 48)

    return nc


def _prep_in_maps(atom_types, embedding):
    import ml_dtypes

    at = np.asarray(atom_types).astype(np.int32).reshape(-1)
    emb = np.asarray(embedding).astype(np.float32)

    # rank-remap: table2[x] = embedding[rank(x)] where rank(x) counts the
    # distinct values < x present anywhere in the batch (identity when all
    # NUM_TYPES values appear).
    present = np.zeros(NUM_TYPES, dtype=bool)
    present[at] = True
    rank = np.cumsum(present) - present
    table2 = emb[np.minimum(rank, NUM_TYPES - 1)].astype(np.float32)
    table2[~present] = 0.0

    iota_in = np.arange(128, dtype=np.float32).reshape(128, 1)
    tbl_in = np.zeros((128, 64), np.float32)
    tbl_in[:100] = table2

    in_maps = []
    for c in range(N_CORES):
        shard = at[c * ROWS_PER_CORE : (c + 1) * ROWS_PER_CORE]
        sp = np.concatenate(
            [shard, np.full(PAD_ROWS - ROWS_PER_CORE, shard[0], np.int32)]
        )
        # atom (p, t) = sp[p*T + t]; chunk k covers slots 4k..4k+3;
        # within-chunk position q = s*128 + p.
        grid = sp.reshape(128, T_SLOTS)  # [p, t]
        feed = grid.reshape(128, N_CHUNKS, 4).transpose(1, 2, 0)  # [k, s, p]
        in_maps.append(
            {
                "idx": np.tile(feed.reshape(1, -1).astype(np.int8), (N_COPIES, 1)),
                "iota": iota_in,
                "tbl": tbl_in.astype(ml_dtypes.bfloat16),
            }
        )
    return in_maps


def run(atom_types, embedding, trace=False):
    from concourse.bass_utils import run_bass_kernel_spmd

    if "nc" not in _CACHE:
        _CACHE["nc"] = _build_graph()
    nc = _CACHE["nc"]

    in_maps = _prep_in_maps(atom_types, embedding)
    res = run_bass_kernel_spmd(
        nc, in_maps, core_ids=list(range(N_CORES)), trace=trace
    )
    shards = [
        np.asarray(r["out"][:ROWS_PER_CORE], dtype=np.float32) for r in res.results
    ]
    full = np.concatenate(shards, axis=0).reshape(N_BATCH, ATOMS_PER_MOL, EMBED_DIM)
    return np.ascontiguousarray(full, dtype=np.float32), res


def kernel(atom_types, embedding):
    out, _ = run(atom_types, embedding, trace=False)
    return out
